# revision 80
# baseline (speedup 1.0000x reference)
"""Trainium2 Bass kernel for nn_Fine_Change_Moment3.

Math (from the reference):
  - input (16,512,512,16) [b,y,x,t]; fc_weight3 (262144,16,6) per-patch 16x6.
  - Only channel 0 of the CAM survives (cam[:, 0]), so only
    fc_weight3[:, :, 0] matters (host-sliced).
  - Per 4x4 patch n=(b,gy,gx): cam0[t] = sum_k patch[k,t] * w[n,k]
  - v = (cam0 - min_t) / max_t(cam0 - min_t)
  - top[b,t] = v arranged (gy,gx); up = A @ top @ A^T with A the 128->512
    bilinear (align_corners) interp matrix; output (b*512*512, 16) f32.

Distribution: data-parallel over batch, 2 batches per core, 8 cores.

Per-core pipeline (b0 = first batch, b1 = second):
  1. DMA input rows contiguously: tiles [y=128][(x256,t16)=4096]
  2. DVE: multiply by per-patch weights (w broadcast over t) -> f16 products
  3. PE: 0/1 selection matmuls with px-strided rhs reduce px AND py across
     partitions and regroup gy -> cam[gy=128][(gx,t)=2048] in PSUM (8
     accumulating matmuls per 512-col chunk per y-tile)
  4. DVE: min/max normalize over t, reading cam straight from PSUM
  5. PE: 16x transpose 128x128 (per t) -> topT[gx][(t,gy)]
  6. PE f16: M1[gy][sx] = topT_t^T @ A^T per t, stored interleaved
     M1i[gy][(sx,t)]; then up[sy][(sx32,t16)] = (A^T chunk)^T @ M1i chunk
  7. copies PSUM->SBUF staging [sy][(x,t)] (t-interleaved), contiguous DMA
     out on the Activation HWDGE queue (reads own the Sync queue).

Emission interleaves b0's (write-paced) upsample groups with b1's stage-2
tiles so the in-order PE stream never head-of-line blocks b1's compute.
"""

import numpy as np

B, S, T, PP = 16, 512, 16, 4
G = S // PP          # 128 patch grid
NCORES = 8
BPC = B // NCORES    # 2 batches per core

_CACHE = {}


def _interp_matrix_np(n_in, n_out):
    # mirrors the reference's align_corners=True bilinear matrix
    coords = np.arange(n_out, dtype=np.float32) * ((n_in - 1) / (n_out - 1))
    i0 = np.clip(np.floor(coords).astype(np.int64), 0, n_in - 2)
    w = coords - i0.astype(np.float32)
    A = np.zeros((n_out, n_in), dtype=np.float32)
    rows = np.arange(n_out)
    np.add.at(A, (rows, i0), 1.0 - w)
    np.add.at(A, (rows, i0 + 1), w)
    return A  # (n_out, n_in)


def _build_program():
    from contextlib import ExitStack
    import concourse.bacc as bacc
    import concourse.tile as tile
    import concourse.mybir as mybir

    f32 = mybir.dt.float32
    f16 = mybir.dt.float16
    Alu = mybir.AluOpType
    Ax = mybir.AxisListType

    nc = bacc.Bacc("TRN2", target_bir_lowering=False, debug=False,
                   num_devices=NCORES)

    x_d = nc.dram_tensor("x", [BPC, S, S, T], f32, kind="ExternalInput")
    w_d = nc.dram_tensor("w", [BPC, 4, 128, 512], f16, kind="ExternalInput")
    at_d = nc.dram_tensor("at", [128, 512], f16, kind="ExternalInput")
    sel_d = nc.dram_tensor("sel", [128, 512], f16, kind="ExternalInput")
    id_d = nc.dram_tensor("ident", [128, 128], f16, kind="ExternalInput")
    # device writes f16 (well inside the 2e-2 tolerance for values in
    # [0,1]); the host upcasts to f32 — halves the HBM write traffic
    y_d = nc.dram_tensor("y", [BPC, S, S, T], f16, kind="ExternalOutput")

    # input view: [b][yt][xh][y_row=128][(x256 t16)=4096]
    x_v = x_d.ap().rearrange("b (yt p) (xh xx) t -> b yt xh p (xx t)",
                             p=128, xh=2)
    # output view: [b][syc][xh][sy=128][(xx256 t16)=4096]
    y_v = y_d.ap().rearrange("b (syc sy) (xh xx) t -> b syc xh sy (xx t)",
                             syc=4, xh=2)

    with tile.TileContext(nc) as tc, ExitStack() as ctx:
        consts = ctx.enter_context(tc.tile_pool(name="consts", bufs=1))
        pin = ctx.enter_context(tc.tile_pool(name="pin", bufs=5))
        pw = ctx.enter_context(tc.tile_pool(name="pw", bufs=4))
        pp1 = ctx.enter_context(tc.tile_pool(name="pp1", bufs=4))
        ps2 = ctx.enter_context(tc.tile_pool(name="ps2", bufs=4))
        pv = ctx.enter_context(tc.tile_pool(name="pv", bufs=2))
        ptop = ctx.enter_context(tc.tile_pool(name="ptop", bufs=1))
        pm1 = ctx.enter_context(tc.tile_pool(name="pm1", bufs=1))
        pst = ctx.enter_context(tc.tile_pool(name="pst", bufs=4))
        ppsc = ctx.enter_context(tc.tile_pool(name="ppsc", bufs=1,
                                              space="PSUM"))
        ppsw = ctx.enter_context(tc.tile_pool(name="ppsw", bufs=2,
                                              space="PSUM"))

        at_sb = consts.tile([128, 512], f16)
        sel_sb = consts.tile([128, 512], f16)
        id_sb = consts.tile([128, 128], f16)
        # consts ride the write-side (Activation) HWDGE queue so the first
        # input tiles own the read queue from t=0
        nc.scalar.dma_start(sel_sb[:], sel_d.ap())
        nc.scalar.dma_start(at_sb[:], at_d.ap())
        nc.scalar.dma_start(id_sb[:], id_d.ap())

        wsb = [None]
        state = {}
        pending = []

        def stage2_tile(b, yt, xh, cam_ps):
            # DMA one [128 rows x (256x,16t)] tile, weight it on DVE (f16
            # products), then 8 selection matmuls (2 chunks x 4 px) fold the
            # px-sum and the py partition-reduction into PSUM accumulation.
            if xh == 0:
                wsb[0] = pw.tile([128, 512], f16, tag="w", name="w_sb")
                nc.sync.dma_start(wsb[0][:], w_d.ap()[b, yt])
            w_sb = wsb[0]
            it = pin.tile([128, 4096], f32, tag="in")
            nc.sync.dma_start(it[:], x_v[b, yt, xh])
            itv = it[:].rearrange("p (gx px t) -> p gx px t", px=PP, t=T)
            wv = (w_sb[:, xh * 256:(xh + 1) * 256]
                  .rearrange("p (gx px) -> p gx px", px=PP)
                  .unsqueeze(3).broadcast_to([128, 64, PP, T]))
            p1 = pp1.tile([128, 4096], f16, tag="p1")
            p1v = p1[:].rearrange("p (gx px t) -> p gx px t", px=PP, t=T)
            nc.vector.tensor_tensor(p1v, itv, wv, op=Alu.mult)
            # one f16 pairwise px-add rides DVE's read-pacing slack
            # (~1.2us/tile); the remaining pair folds into the PE selection
            # matmuls, halving their count vs a full px fold
            pr = p1[:].rearrange("p (gx pxp px2 t) -> p gx pxp px2 t",
                                 pxp=2, px2=2, t=T)
            s2 = ps2.tile([128, 2048], f16, tag="s2")
            s2v = s2[:].rearrange("p (gx pxp t) -> p gx pxp t", pxp=2, t=T)
            nc.vector.tensor_tensor(s2v, pr[:, :, :, 0, :],
                                    pr[:, :, :, 1, :], op=Alu.add)
            for fc in (0, 1):
                f = xh * 2 + fc
                for j in range(2):
                    nc.tensor.matmul(
                        cam_ps[:, f * 512:(f + 1) * 512],
                        lhsT=sel_sb[:, yt * 128:(yt + 1) * 128],
                        rhs=s2v[:, fc * 32:(fc + 1) * 32, j, :],
                        start=(yt == 0 and j == 0),
                        stop=(yt == 3 and j == 1),
                    )

        def norm_tp_m1(b, cam_ps):
            tailb = (b == BPC - 1)
            # ---- normalize over t per (gy, gx), reading cam from PSUM
            v = pv.tile([128, 2048], f16, tag="v")
            mn = pv.tile([128, 128], f32, tag="mn")
            mx = pv.tile([128, 128], f32, tag="mx")
            rx = pv.tile([128, 128], f32, tag="rx")
            cam3 = cam_ps[:].rearrange("p (gx t) -> p gx t", t=T)
            v3 = v[:].rearrange("p (gx t) -> p gx t", t=T)
            nc.vector.tensor_reduce(mn[:], cam3, axis=Ax.X, op=Alu.min)
            mnb = mn[:].unsqueeze(2).broadcast_to([128, 128, T])
            nc.vector.tensor_tensor(v3, cam3, mnb, op=Alu.subtract)
            nc.vector.tensor_reduce(mx[:], v3, axis=Ax.X, op=Alu.max)
            nc.vector.reciprocal(rx[:], mx[:])
            rxb = rx[:].unsqueeze(2).broadcast_to([128, 128, T])
            if tailb:
                # split the final scale by t-halves: the first transposes
                # (t<8) overlap the second half of the multiply
                nc.vector.tensor_tensor(v3[:, :, :8], v3[:, :, :8],
                                        rxb[:, :, :8], op=Alu.mult)
                nc.vector.tensor_tensor(v3[:, :, 8:], v3[:, :, 8:],
                                        rxb[:, :, 8:], op=Alu.mult)
            else:
                nc.vector.tensor_tensor(v3, v3, rxb, op=Alu.mult)

            # ---- per-t 128x128 transposes -> topT[gx][(t,gy)]
            topT = ptop.tile([128, 2048], f16, tag="top")
            vt = v[:].rearrange("p (gx t) -> p t gx", t=T)
            tp_ps = ppsw.tile([128, 2048], f16, tag="pw", name="tp_ps")
            for t in range(T):
                nc.tensor.transpose(tp_ps[:, t * 128:(t + 1) * 128],
                                    vt[:, t, :], id_sb[:])
            if tailb:
                nc.scalar.copy(topT[:, :1024], tp_ps[:, :1024])
                nc.vector.tensor_copy(topT[:, 1024:], tp_ps[:, 1024:])
            else:
                nc.scalar.copy(topT[:], tp_ps[:])

            # ---- M1 per t -> m1i[gy][(sx,t)] f16, pair-strided
            m1i = pm1.tile([128, 8192], f16, tag="m1i")
            m1iv = m1i[:].rearrange("p (sx t) -> p sx t", t=T)
            for tq in range(8):
                if tailb and tq % 3 == 2:
                    m1_ps = ppsc.tile([128, 1024], f32, tag="cam",
                                      name="m1_ps")
                else:
                    m1_ps = ppsw.tile([128, 1024], f32, tag="pw",
                                      name="m1_ps")
                for tl in range(2):
                    t = tq * 2 + tl
                    nc.tensor.matmul(
                        m1_ps[:, tl * 512:(tl + 1) * 512],
                        lhsT=topT[:, t * 128:(t + 1) * 128],
                        rhs=at_sb[:],
                        start=True, stop=True,
                    )
                csrc = m1_ps[:].rearrange("p (tl sx) -> p sx tl", tl=2)
                cdst = m1iv[:, :, tq * 2:(tq + 1) * 2]
                if tailb:
                    # halve the per-tq copy latency: ACT and DVE each take
                    # half the sx range, in parallel
                    nc.scalar.copy(cdst[:, :256], csrc[:, :256])
                    nc.vector.tensor_copy(cdst[:, 256:], csrc[:, 256:])
                else:
                    nc.scalar.copy(cdst, csrc)
            state[b] = m1i

        def up_group(b, g):
            # one output stripe [sy=128][(x256,t16)]: 8 matmuls + 4 copies,
            # staged contiguously and written on the Activation HWDGE queue
            tailb = (b == BPC - 1)
            syc, xh = divmod(g, 2)
            m1i = state[b]
            stg = pst.tile([128, 4096], f16, tag="stg")
            for sxg in range(4):
                gi = g * 4 + sxg
                if tailb and gi % 3 == 2:
                    up_ps = ppsc.tile([128, 1024], f32, tag="cam")
                else:
                    up_ps = ppsw.tile([128, 1024], f32, tag="pw")
                for sxl in range(2):
                    sxblk = (xh * 4 + sxg) * 2 + sxl
                    nc.tensor.matmul(
                        up_ps[:, sxl * 512:(sxl + 1) * 512],
                        lhsT=at_sb[:, syc * 128:(syc + 1) * 128],
                        rhs=m1i[:, sxblk * 512:(sxblk + 1) * 512],
                        start=True, stop=True,
                    )
                dst = stg[:, sxg * 1024:(sxg + 1) * 1024]
                if tailb and sxg % 2 == 1:
                    nc.vector.tensor_copy(dst, up_ps[:])
                else:
                    nc.scalar.copy(dst, up_ps[:])
                if tailb and sxg == 1:
                    # last batch: ship each staged half as soon as it is
                    # ready, dispatched from the idle Sync engine so the
                    # Activation engine stays on copies
                    nc.sync.dma_start(y_v[b, syc, xh][:, :2048],
                                      stg[:, :2048])
            if tailb:
                nc.sync.dma_start(y_v[b, syc, xh][:, 2048:],
                                  stg[:, 2048:])
            elif g < 4:
                nc.scalar.dma_start(y_v[b, syc, xh], stg[:])
            else:
                # b0's last stripes: dispatch later on the Sync queue,
                # behind all of b1's reads (same-queue FIFO = the reads
                # never lose bandwidth to them), draining into the DMA
                # idle window while b1's tail computes
                pending.append((stg, syc, xh))

        # ---- emission: b0 front half, then b0's upsample interleaved with
        # b1's stage-2 tiles (keeps the in-order PE stream from blocking),
        # then b1's tail.
        cam0 = ppsc.tile([128, 2048], f32, tag="cam")
        for yt in range(4):
            for xh in range(2):
                stage2_tile(0, yt, xh, cam0)
        norm_tp_m1(0, cam0)

        cam1 = ppsc.tile([128, 2048], f32, tag="cam")
        for g in range(8):
            up_group(0, g)
            yt, xh = divmod(g, 2)
            stage2_tile(1, yt, xh, cam1)
        for stgp, syc, xh in pending:
            nc.sync.dma_start(y_v[0, syc, xh], stgp[:])
        norm_tp_m1(1, cam1)
        for g in range(8):
            up_group(1, g)

    nc.compile()
    return nc


def _host_prep(input, fc_weight3):
    inp = np.ascontiguousarray(input, dtype=np.float32)
    w0 = np.ascontiguousarray(fc_weight3[:, :, 0], dtype=np.float32)
    # w0: (N,16) with n=(b,gy,gx), k=(py,px)
    w0 = w0.reshape(B, 4, 32, G, PP, PP)          # b yt gy_l gx py px
    w_arr = np.ascontiguousarray(
        w0.transpose(0, 1, 2, 4, 3, 5).reshape(B, 4, 128, 512)
        .astype(np.float16))

    A = _interp_matrix_np(G, S)                   # (512,128)
    at = np.ascontiguousarray(A.T.astype(np.float16))  # (128,512)

    sel = np.zeros((128, 512), dtype=np.float16)
    p = np.arange(128)
    for j in range(4):
        sel[p, j * 128 + 32 * j + p // 4] = 1.0

    ident = np.eye(128, dtype=np.float16)
    return inp, w_arr, at, sel, ident


def kernel(input, fc_weight3):
    from concourse.bass_utils import run_bass_kernel_spmd

    if "nc" not in _CACHE:
        _CACHE["nc"] = _build_program()
    nc = _CACHE["nc"]

    inp, w_arr, at, sel, ident = _host_prep(input, fc_weight3)

    in_maps = []
    for c in range(NCORES):
        in_maps.append({
            "x": inp[c * BPC:(c + 1) * BPC],
            "w": w_arr[c * BPC:(c + 1) * BPC],
            "at": at,
            "sel": sel,
            "ident": ident,
        })
    res = run_bass_kernel_spmd(nc, in_maps, list(range(NCORES)))
    out = np.concatenate([r["y"] for r in res.results], axis=0)
    return out.reshape(-1, T).astype(np.float32)


# revision 82
# speedup vs baseline: 1.2226x; 1.2226x over previous
"""Trainium2 Bass kernel for nn_Fine_Change_Moment3.

Math (from the reference):
  - input (16,512,512,16) [b,y,x,t]; fc_weight3 (262144,16,6) per-patch 16x6.
  - Only channel 0 of the CAM survives (cam[:, 0]), so only
    fc_weight3[:, :, 0] matters (host-sliced).
  - Per 4x4 patch n=(b,gy,gx): cam0[t] = sum_k patch[k,t] * w[n,k]
  - v = (cam0 - min_t) / max_t(cam0 - min_t)
  - top[b,t] = v arranged (gy,gx); up = A @ top @ A^T with A the 128->512
    bilinear (align_corners) interp matrix; output (b*512*512, 16) f32.

Distribution: data-parallel over batch, 2 batches per core, 8 cores.

Per-core pipeline (b0 = first batch, b1 = second):
  1. DMA input rows contiguously: tiles [y=128][(x256,t16)=4096]
  2. DVE: multiply by per-patch weights (w broadcast over t) -> f16 products
  3. PE: 0/1 selection matmuls with px-strided rhs reduce px AND py across
     partitions and regroup gy -> cam[gy=128][(gx,t)=2048] in PSUM (8
     accumulating matmuls per 512-col chunk per y-tile)
  4. DVE: min/max normalize over t, reading cam straight from PSUM
  5. PE: 16x transpose 128x128 (per t) -> topT[gx][(t,gy)]
  6. PE f16: M1[gy][sx] = topT_t^T @ A^T per t, stored interleaved
     M1i[gy][(sx,t)]; then up[sy][(sx32,t16)] = (A^T chunk)^T @ M1i chunk
  7. copies PSUM->SBUF staging [sy][(x,t)] (t-interleaved), contiguous DMA
     out on the Activation HWDGE queue (reads own the Sync queue).

Emission interleaves b0's (write-paced) upsample groups with b1's stage-2
tiles so the in-order PE stream never head-of-line blocks b1's compute.
"""

import numpy as np

B, S, T, PP = 16, 512, 16, 4
G = S // PP          # 128 patch grid
NCORES = 8
BPC = B // NCORES    # 2 batches per core

_CACHE = {}


def _interp_matrix_np(n_in, n_out):
    # mirrors the reference's align_corners=True bilinear matrix
    coords = np.arange(n_out, dtype=np.float32) * ((n_in - 1) / (n_out - 1))
    i0 = np.clip(np.floor(coords).astype(np.int64), 0, n_in - 2)
    w = coords - i0.astype(np.float32)
    A = np.zeros((n_out, n_in), dtype=np.float32)
    rows = np.arange(n_out)
    np.add.at(A, (rows, i0), 1.0 - w)
    np.add.at(A, (rows, i0 + 1), w)
    return A  # (n_out, n_in)


def _build_program():
    from contextlib import ExitStack
    import concourse.bacc as bacc
    import concourse.tile as tile
    import concourse.mybir as mybir

    f32 = mybir.dt.float32
    f16 = mybir.dt.float16
    Alu = mybir.AluOpType
    Ax = mybir.AxisListType

    nc = bacc.Bacc("TRN2", target_bir_lowering=False, debug=False,
                   num_devices=NCORES)

    x_d = nc.dram_tensor("x", [BPC, S, S, T], f32, kind="ExternalInput")
    w_d = nc.dram_tensor("w", [BPC, 4, 128, 512], f16, kind="ExternalInput")
    at_d = nc.dram_tensor("at", [128, 512], f16, kind="ExternalInput")
    sel_d = nc.dram_tensor("sel", [128, 512], f16, kind="ExternalInput")
    id_d = nc.dram_tensor("ident", [128, 128], f16, kind="ExternalInput")
    # device writes f16 (well inside the 2e-2 tolerance for values in
    # [0,1]); the host upcasts to f32 — halves the HBM write traffic
    y_d = nc.dram_tensor("y", [BPC, S, S, T], f16, kind="ExternalOutput")

    # input view: [b][yt][xh][y_row=128][(x256 t16)=4096]
    x_v = x_d.ap().rearrange("b (yt p) (xh xx) t -> b yt xh p (xx t)",
                             p=128, xh=2)
    # output view: [b][syc][xh][sy=128][(xx256 t16)=4096]
    y_v = y_d.ap().rearrange("b (syc sy) (xh xx) t -> b syc xh sy (xx t)",
                             syc=4, xh=2)

    with tile.TileContext(nc) as tc, ExitStack() as ctx:
        consts = ctx.enter_context(tc.tile_pool(name="consts", bufs=1))
        pin = ctx.enter_context(tc.tile_pool(name="pin", bufs=6))
        pw = ctx.enter_context(tc.tile_pool(name="pw", bufs=4))
        # p1 is produced and consumed back-to-back on DVE (program order),
        # so 2 bufs suffice; the freed SBUF deepens the input prefetch
        pp1 = ctx.enter_context(tc.tile_pool(name="pp1", bufs=2))
        ps2 = ctx.enter_context(tc.tile_pool(name="ps2", bufs=4))
        pv = ctx.enter_context(tc.tile_pool(name="pv", bufs=2))
        ptop = ctx.enter_context(tc.tile_pool(name="ptop", bufs=1))
        pm1 = ctx.enter_context(tc.tile_pool(name="pm1", bufs=1))
        pst = ctx.enter_context(tc.tile_pool(name="pst", bufs=4))
        ppsc = ctx.enter_context(tc.tile_pool(name="ppsc", bufs=1,
                                              space="PSUM"))
        ppsw = ctx.enter_context(tc.tile_pool(name="ppsw", bufs=2,
                                              space="PSUM"))

        at_sb = consts.tile([128, 512], f16)
        sel_sb = consts.tile([128, 512], f16)
        id_sb = consts.tile([128, 128], f16)
        # consts ride the write-side (Activation) HWDGE queue so the first
        # input tiles own the read queue from t=0
        nc.scalar.dma_start(sel_sb[:], sel_d.ap())
        nc.scalar.dma_start(at_sb[:], at_d.ap())
        nc.scalar.dma_start(id_sb[:], id_d.ap())

        wsb = [None]
        state = {}

        def stage2_tile(b, yt, xh, cam_ps):
            # DMA one [128 rows x (256x,16t)] tile, weight it on DVE (f16
            # products), then 8 selection matmuls (2 chunks x 4 px) fold the
            # px-sum and the py partition-reduction into PSUM accumulation.
            if xh == 0:
                wsb[0] = pw.tile([128, 512], f16, tag="w", name="w_sb")
                nc.sync.dma_start(wsb[0][:], w_d.ap()[b, yt])
            w_sb = wsb[0]
            it = pin.tile([128, 4096], f32, tag="in")
            nc.sync.dma_start(it[:], x_v[b, yt, xh])
            itv = it[:].rearrange("p (gx px t) -> p gx px t", px=PP, t=T)
            wv = (w_sb[:, xh * 256:(xh + 1) * 256]
                  .rearrange("p (gx px) -> p gx px", px=PP)
                  .unsqueeze(3).broadcast_to([128, 64, PP, T]))
            p1 = pp1.tile([128, 4096], f16, tag="p1")
            p1v = p1[:].rearrange("p (gx px t) -> p gx px t", px=PP, t=T)
            nc.vector.tensor_tensor(p1v, itv, wv, op=Alu.mult)
            # one f16 pairwise px-add rides DVE's read-pacing slack
            # (~1.2us/tile); the remaining pair folds into the PE selection
            # matmuls, halving their count vs a full px fold
            pr = p1[:].rearrange("p (gx pxp px2 t) -> p gx pxp px2 t",
                                 pxp=2, px2=2, t=T)
            s2 = ps2.tile([128, 2048], f16, tag="s2")
            s2v = s2[:].rearrange("p (gx pxp t) -> p gx pxp t", pxp=2, t=T)
            nc.vector.tensor_tensor(s2v, pr[:, :, :, 0, :],
                                    pr[:, :, :, 1, :], op=Alu.add)
            for fc in (0, 1):
                f = xh * 2 + fc
                for j in range(2):
                    nc.tensor.matmul(
                        cam_ps[:, f * 512:(f + 1) * 512],
                        lhsT=sel_sb[:, yt * 128:(yt + 1) * 128],
                        rhs=s2v[:, fc * 32:(fc + 1) * 32, j, :],
                        start=(yt == 0 and j == 0),
                        stop=(yt == 3 and j == 1),
                    )

        def norm_tp_m1(b, cam_ps):
            tailb = (b == BPC - 1)
            # ---- normalize over t per (gy, gx), reading cam from PSUM
            v = pv.tile([128, 2048], f16, tag="v")
            mn = pv.tile([128, 128], f32, tag="mn")
            mx = pv.tile([128, 128], f32, tag="mx")
            rx = pv.tile([128, 128], f32, tag="rx")
            cam3 = cam_ps[:].rearrange("p (gx t) -> p gx t", t=T)
            v3 = v[:].rearrange("p (gx t) -> p gx t", t=T)
            nc.vector.tensor_reduce(mn[:], cam3, axis=Ax.X, op=Alu.min)
            mnb = mn[:].unsqueeze(2).broadcast_to([128, 128, T])
            nc.vector.tensor_tensor(v3, cam3, mnb, op=Alu.subtract)
            nc.vector.tensor_reduce(mx[:], v3, axis=Ax.X, op=Alu.max)
            nc.vector.reciprocal(rx[:], mx[:])
            rxb = rx[:].unsqueeze(2).broadcast_to([128, 128, T])
            if tailb:
                # split the final scale by t-halves: the first transposes
                # (t<8) overlap the second half of the multiply
                nc.vector.tensor_tensor(v3[:, :, :8], v3[:, :, :8],
                                        rxb[:, :, :8], op=Alu.mult)
                nc.vector.tensor_tensor(v3[:, :, 8:], v3[:, :, 8:],
                                        rxb[:, :, 8:], op=Alu.mult)
            else:
                nc.vector.tensor_tensor(v3, v3, rxb, op=Alu.mult)

            # ---- per-t 128x128 transposes -> topT[gx][(t,gy)]
            topT = ptop.tile([128, 2048], f16, tag="top")
            vt = v[:].rearrange("p (gx t) -> p t gx", t=T)
            tp_ps = ppsw.tile([128, 2048], f16, tag="pw", name="tp_ps")
            for t in range(T):
                nc.tensor.transpose(tp_ps[:, t * 128:(t + 1) * 128],
                                    vt[:, t, :], id_sb[:])
            if tailb:
                nc.scalar.copy(topT[:, :1024], tp_ps[:, :1024])
                nc.vector.tensor_copy(topT[:, 1024:], tp_ps[:, 1024:])
            else:
                nc.scalar.copy(topT[:], tp_ps[:])

            # ---- M1 per t -> m1i[gy][(sx,t)] f16, pair-strided
            m1i = pm1.tile([128, 8192], f16, tag="m1i")
            m1iv = m1i[:].rearrange("p (sx t) -> p sx t", t=T)
            for tq in range(8):
                if tailb and tq % 3 == 2:
                    m1_ps = ppsc.tile([128, 1024], f32, tag="cam",
                                      name="m1_ps")
                else:
                    m1_ps = ppsw.tile([128, 1024], f32, tag="pw",
                                      name="m1_ps")
                for tl in range(2):
                    t = tq * 2 + tl
                    nc.tensor.matmul(
                        m1_ps[:, tl * 512:(tl + 1) * 512],
                        lhsT=topT[:, t * 128:(t + 1) * 128],
                        rhs=at_sb[:],
                        start=True, stop=True,
                    )
                csrc = m1_ps[:].rearrange("p (tl sx) -> p sx tl", tl=2)
                cdst = m1iv[:, :, tq * 2:(tq + 1) * 2]
                if tailb:
                    # halve the per-tq copy latency: ACT and DVE each take
                    # half the sx range, in parallel
                    nc.scalar.copy(cdst[:, :256], csrc[:, :256])
                    nc.vector.tensor_copy(cdst[:, 256:], csrc[:, 256:])
                else:
                    nc.scalar.copy(cdst, csrc)
            state[b] = m1i

        def up_group(b, g):
            # one output stripe [sy=128][(x256,t16)]: 8 matmuls + 4 copies,
            # staged contiguously and written on the Activation HWDGE queue
            tailb = (b == BPC - 1)
            syc, xh = divmod(g, 2)
            m1i = state[b]
            stg = pst.tile([128, 4096], f16, tag="stg")
            for sxg in range(4):
                gi = g * 4 + sxg
                if tailb and gi % 3 == 2:
                    up_ps = ppsc.tile([128, 1024], f32, tag="cam")
                else:
                    up_ps = ppsw.tile([128, 1024], f32, tag="pw")
                for sxl in range(2):
                    sxblk = (xh * 4 + sxg) * 2 + sxl
                    nc.tensor.matmul(
                        up_ps[:, sxl * 512:(sxl + 1) * 512],
                        lhsT=at_sb[:, syc * 128:(syc + 1) * 128],
                        rhs=m1i[:, sxblk * 512:(sxblk + 1) * 512],
                        start=True, stop=True,
                    )
                dst = stg[:, sxg * 1024:(sxg + 1) * 1024]
                if tailb and sxg % 2 == 1:
                    nc.vector.tensor_copy(dst, up_ps[:])
                else:
                    nc.scalar.copy(dst, up_ps[:])
                if tailb and sxg == 1:
                    # last batch: ship each staged half as soon as it is
                    # ready, dispatched from the idle Sync engine so the
                    # Activation engine stays on copies
                    nc.sync.dma_start(y_v[b, syc, xh][:, :2048],
                                      stg[:, :2048])
            if tailb:
                nc.sync.dma_start(y_v[b, syc, xh][:, 2048:],
                                  stg[:, 2048:])
            else:
                nc.scalar.dma_start(y_v[b, syc, xh], stg[:])

        # ---- emission: b0 front half, then b0's upsample interleaved with
        # b1's stage-2 tiles (keeps the in-order PE stream from blocking),
        # then b1's tail.
        cam0 = ppsc.tile([128, 2048], f32, tag="cam")
        for yt in range(4):
            for xh in range(2):
                stage2_tile(0, yt, xh, cam0)
        norm_tp_m1(0, cam0)

        cam1 = ppsc.tile([128, 2048], f32, tag="cam")
        for g in range(8):
            up_group(0, g)
            yt, xh = divmod(g, 2)
            stage2_tile(1, yt, xh, cam1)
        norm_tp_m1(1, cam1)
        for g in range(8):
            up_group(1, g)

    nc.compile()
    return nc


def _host_prep(input, fc_weight3):
    inp = np.ascontiguousarray(input, dtype=np.float32)
    w0 = np.ascontiguousarray(fc_weight3[:, :, 0], dtype=np.float32)
    # w0: (N,16) with n=(b,gy,gx), k=(py,px)
    w0 = w0.reshape(B, 4, 32, G, PP, PP)          # b yt gy_l gx py px
    w_arr = np.ascontiguousarray(
        w0.transpose(0, 1, 2, 4, 3, 5).reshape(B, 4, 128, 512)
        .astype(np.float16))

    A = _interp_matrix_np(G, S)                   # (512,128)
    at = np.ascontiguousarray(A.T.astype(np.float16))  # (128,512)

    sel = np.zeros((128, 512), dtype=np.float16)
    p = np.arange(128)
    for j in range(4):
        sel[p, j * 128 + 32 * j + p // 4] = 1.0

    ident = np.eye(128, dtype=np.float16)
    return inp, w_arr, at, sel, ident


def kernel(input, fc_weight3):
    from concourse.bass_utils import run_bass_kernel_spmd

    if "nc" not in _CACHE:
        _CACHE["nc"] = _build_program()
    nc = _CACHE["nc"]

    inp, w_arr, at, sel, ident = _host_prep(input, fc_weight3)

    in_maps = []
    for c in range(NCORES):
        in_maps.append({
            "x": inp[c * BPC:(c + 1) * BPC],
            "w": w_arr[c * BPC:(c + 1) * BPC],
            "at": at,
            "sel": sel,
            "ident": ident,
        })
    res = run_bass_kernel_spmd(nc, in_maps, list(range(NCORES)))
    out = np.concatenate([r["y"] for r in res.results], axis=0)
    return out.reshape(-1, T).astype(np.float32)
